# revision 45
# baseline (speedup 1.0000x reference)
"""Multi-head attention kernel for Trainium2, sharded one head per NeuronCore.

Math (per head h, batch b):
  q,k,v = W_{q,k,v} @ x        [32, n]   (n = 48*48 = 2304, c = 256)
  S~[j,i] = sum_d k[d,j] q[d,i]          (S transposed: j on partitions)
  P[j,i]  = exp(S~[j,i] + pos_bias[h].T[j,i])
     computed either as exp(S~)*exp(B) (Act exp + DVE mult) or via the
     Schraudolph bit-trick on DVE/GpSimd:
       bf16bits(P) ~= int16(A*S~ + round(A*B + 16256)),  A = 128/ln2
     which fuses the bias add and the exp into one off-Act instruction.
  O_ext: [v.T | 1]^T-contracted with P, 2-way column-tiled over the PE
     (even j-chunks -> psum rows 0..32, odd -> rows 64..96) so the two
     accumulation chains run concurrently on distinct col groups:
       rows 0..31/64..95 = partial attn@v (transposed), rows 32/96 = sums
  out_un[c,i] = sum_r wo2[c, r] * o_t[r, i]   (wo2 = [wo; 0; wo; 0])
Host: out = sum_h out_un_h / (sums0_h + sums1_h) + b_out.
"""

import sys

for _p in ("/opt/trn_rl_repo", "/root/.axon_site/_ro/trn_rl_repo"):
    if _p not in sys.path:
        sys.path.append(_p)

import numpy as np
import ml_dtypes

import concourse.bacc as bacc
import concourse.mybir as mybir
import concourse.tile as tile
from concourse import bass_utils

HEADS = 8
D = 32                      # dim per head
SCALE = D ** -0.5
B = 4                       # batch
C = 256                     # channels
N = 2304                    # tokens (48*48)
H = W = 48
NJ = 18                     # 128-row j-chunks
JG = 3                      # j-chunks per ACT group (3 psum banks)
NG = NJ // JG               # groups per (b, i-block)
IBLOCKS = [(0, 512), (512, 512), (1024, 512), (1536, 512), (2048, 256)]

F32 = mybir.dt.float32
F32R = mybir.dt.float32r
BF16 = mybir.dt.bfloat16
I16 = mybir.dt.int16
EXP = mybir.ActivationFunctionType.Exp
MULT = mybir.AluOpType.mult
ADD = mybir.AluOpType.add

SCH_A = float(128.0 / np.log(2.0))   # Schraudolph scale: bf16 exponent grid
# 127<<7, mean-centered (-7.37) so mixed exact/approx rows carry no relative
# bias in the softmax, +0.5 compensating the device's truncating f32->i16.
SCH_B = 16256.0 - 7.37 + 0.5

VARIANT = "full"
LAG_M = 3                  # groups of slack: exp -> eb-mult
LAG_O = 12                 # groups of slack: P final -> O matmuls
# closings must emit immediately after their unit's last O matmul: psA
# buffer rotation (2 banks) deadlocks if the next unit's o_ps is allocated
# before the previous closing's op_ps.
LAG_C = LAG_O
# tuning knobs
OUT_EVAC_ENG = "scalar"    # engine for out-projection psum->sbuf evacuation
O_COLTILE = True           # 2-way col-tiled O accumulation
# per (ib, g) group path: 'A' = Act exp + DVE mult, 'B' = Act exp + Pool
# (gpsimd) mult, 'D' = DVE schraudolph (GPSIMD cannot read PSUM, so the
# schraudolph path is DVE-only).  B groups lead each i-block: their
# two-engine chain has the longest latency, so give them the most slack
# before their deferred O matmuls reach the head of the PE queue.
ASSIGN = (
    "BDBDAA",
    "BDADAD",
    "BDBDAA",
    "BDADAD",
    "BDA",
)
QK_EVAC_ENG = "vector"     # engine for q/k psum->sbuf evacuation
OT_EVAC_ENG = "scalar"     # engine for o_t psum->sbuf evacuation
SPSUM_BUFS = 2             # S-psum group double buffering (3 banks each)
PP_BUFS = 15               # P-tile pool depth


def _emit(nc, reps=1):
    x_d = nc.dram_tensor("x", [B, C, N], BF16, kind="ExternalInput")
    wq_d = nc.dram_tensor("wq", [C, 96], BF16, kind="ExternalInput")
    wk_d = nc.dram_tensor("wk", [C, 96], BF16, kind="ExternalInput")
    wv_d = nc.dram_tensor("wv", [C, D], BF16, kind="ExternalInput")
    wo_d = nc.dram_tensor("wo", [97, C], F32, kind="ExternalInput")
    eb_d = nc.dram_tensor("expb", [N, N], BF16, kind="ExternalInput")
    ebi_d = nc.dram_tensor("ebi", [N, N], I16, kind="ExternalInput")
    out_d = nc.dram_tensor("out_un", [B, C, N], F32, kind="ExternalOutput")
    sums_d = nc.dram_tensor("sums", [B, 2, N], F32, kind="ExternalOutput")

    def path_of(ib, g):
        return ASSIGN[ib][g]

    with tile.TileContext(nc) as tc:
        with (
            tc.tile_pool(name="wpool", bufs=1) as wpool,
            tc.tile_pool(name="qk", bufs=8) as qkpool,
            tc.tile_pool(name="vext", bufs=4) as vpool,
            tc.tile_pool(name="big", bufs=2) as bigpool,
            tc.tile_pool(name="pp", bufs=PP_BUFS) as ppool,
            tc.tile_pool(name="ebpool", bufs=2) as ebpool,
            tc.tile_pool(name="outsb", bufs=3) as outpool,
            tc.tile_pool(name="osb", bufs=2) as opool,
            tc.tile_pool(name="spsum", bufs=SPSUM_BUFS, space="PSUM") as spsum,
            tc.tile_pool(name="psA", bufs=2, space="PSUM") as psA,
        ):
            # ---- weights: bf16 straight from HBM (host pre-converts) ----
            w_r = {}
            for name, dram, shape in (
                ("wq", wq_d, [128, 2, 96]),
                ("wk", wk_d, [128, 2, 96]),
                ("wv", wv_d, [128, 2, D]),
            ):
                raw = wpool.tile(shape, BF16, tag=f"{name}raw")
                nc.sync.dma_start(raw, dram.ap().rearrange("(cc p) m -> p cc m", p=128))
                w_r[name] = raw
            wo_raw = wpool.tile([97, C], F32, tag="woraw")
            nc.sync.dma_start(wo_raw, wo_d.ap())
            wo_r = wpool.tile([97, C], F32R, tag="wor")
            nc.vector.tensor_copy(wo_r, wo_raw)
            zrow = wpool.tile([1, 512], F32, tag="zrow")
            nc.vector.memset(zrow, 0.0)
            # o_t buffers: zero once so the dead partition band (33..63)
            # contributes exact zeros to the K=97 out-projection.
            for _ in range(2):
                t = opool.tile([128, 512], F32R, tag="ot")
                nc.vector.memset(t.bitcast(F32), 0.0)

            # ---- phase 0 per batch: load x, project q/k/v ----
            q_sb = [None] * B
            k_sb = [None] * B
            v_sb = [None] * B

            def proj_batch(b):
                x_r = bigpool.tile([128, 2, N], BF16, tag="big")
                x_view = x_d.ap()[b].rearrange("(cc p) n -> p cc n", p=128)
                for cc in range(2):
                    nc.sync.dma_start(x_r[:, cc, :], x_view[:, cc, :])

                # q and k replicated 3x along output rows (for PE row-tiling)
                for name, store in (("wk", k_sb), ("wq", q_sb)):
                    dst = qkpool.tile([128, N], F32R, tag="qk")
                    store[b] = dst
                    for ti, islices in ((0, (0, 1, 2)), (1, (3, 4))):
                        pt = spsum.tile([128, 3 * 512], F32, tag="sg")
                        for sl, ic in enumerate(islices):
                            i0, iw = IBLOCKS[ic]
                            for cc in range(2):
                                nc.tensor.matmul(
                                    pt[0:96, sl * 512 : sl * 512 + iw],
                                    w_r[name][:, cc, :],
                                    x_r[:, cc, i0 : i0 + iw],
                                    start=(cc == 0),
                                    stop=(cc == 1),
                                )
                        nw = sum(IBLOCKS[ic][1] for ic in islices)
                        eng = nc.scalar if b < 1 else getattr(nc, QK_EVAC_ENG)
                        if eng is nc.scalar:
                            eng.copy(
                                dst[0:96, ti * 1536 : ti * 1536 + nw], pt[0:96, 0:nw]
                            )
                        else:
                            eng.tensor_copy(
                                dst[0:96, ti * 1536 : ti * 1536 + nw], pt[0:96, 0:nw]
                            )

                # v transposed directly: v_T[n, d] = x^T @ wv_T, 18 chunks
                vext = vpool.tile([128, NJ * (D + 1)], BF16, tag="vext")
                v_sb[b] = vext
                nc.vector.memset(vext, 1.0)
                vt = spsum.tile([128, 3 * 512], F32, tag="sg")
                for jc in range(NJ):
                    for cc in range(2):
                        nc.tensor.matmul(
                            vt[:, jc * D : (jc + 1) * D],
                            x_r[:, cc, jc * 128 : (jc + 1) * 128],
                            w_r["wv"][:, cc, :],
                            start=(cc == 0),
                            stop=(cc == 1),
                        )
                nc.vector.tensor_copy(
                    vext.rearrange("p (jc m) -> p jc m", m=D + 1)[:, :, 0:D],
                    vt.rearrange("p (jc m) -> p jc m", m=D)[:, 0:NJ, :],
                )

            # Deferred-emission queues.  Every engine queue is strict FIFO
            # on hardware, so an instruction whose inputs aren't ready
            # head-of-line-blocks everything behind it on that engine.
            # Emit each dependent stage LAG groups after its producer so by
            # the time it reaches its engine's queue head, the input exists:
            #   m_queue: A/B-path eb-multiplies (wait on Act exp)
            #   o_queue: O matmuls (wait on final P)
            #   c_queue: per-(b,ib) closings (wait on all 18 O matmuls)
            m_queue, o_queue, c_queue = [], [], []
            gctr = [0]

            def pump(cur):
                while m_queue and m_queue[0][0] <= cur - LAG_M:
                    m_queue.pop(0)[1]()
                while o_queue and o_queue[0][0] <= cur - LAG_O:
                    o_queue.pop(0)[1]()
                while c_queue and c_queue[0][0] <= cur - LAG_C:
                    c_queue.pop(0)[1]()

            def flush_all():
                # drain in global group order (ties: mult, O, closing) so
                # psA buffer reuse stays emission-ordered
                while m_queue or o_queue or c_queue:
                    cands = []
                    if m_queue:
                        cands.append((m_queue[0][0], 0, m_queue))
                    if o_queue:
                        cands.append((o_queue[0][0], 1, o_queue))
                    if c_queue:
                        cands.append((c_queue[0][0], 2, c_queue))
                    cands.sort()
                    cands[0][2].pop(0)[1]()

            def group_layout(iw):
                """Per ACT-group chunk placement in the 3-bank S tile."""
                if iw == 512:
                    return [[(g * 3 + jl, jl, jl * 512) for jl in range(3)]
                            for g in range(6)]
                return [
                    [(g * 6 + c, c % 3, (c % 3) * 512 + (c // 3) * 256)
                     for c in range(6)]
                    for g in range(3)
                ]

            def attn(b, ib, eb_t):
                i0, iw = IBLOCKS[ib]
                # o_ps is allocated lazily inside the first deferred O-thunk
                # so psA buffer reuse follows emission order (no deadlock).
                o_ps_box = []

                def get_o_ps():
                    if not o_ps_box:
                        o_ps_box.append(
                            psA.tile([128, 512], F32, tag="pa", name="o_ps")
                        )
                    return o_ps_box[0]

                for g, chunks in enumerate(group_layout(iw)):
                    path = path_of(ib, g)
                    s_ps = spsum.tile([128, 3 * 512], F32, tag="sg")
                    for jc, row, off in chunks:
                        nc.tensor.matmul(
                            s_ps[:, off : off + iw],
                            k_sb[b][32 * row : 32 * row + 32, jc * 128 : (jc + 1) * 128],
                            q_sb[b][32 * row : 32 * row + 32, i0 : i0 + iw],
                            start=True,
                            stop=True,
                        )
                    p_t = ppool.tile([128, 3 * 512], BF16, tag="pt")
                    gc = gctr[0]
                    gctr[0] += 1
                    if path == "D":
                        # Schraudolph: bf16bits = int16(A*S + ebi), fused
                        # bias add + exp approx, off the Act engine.
                        nc.vector.scalar_tensor_tensor(
                            p_t.bitcast(I16),
                            s_ps,
                            SCH_A,
                            eb_t.bitcast(I16)[:, g * 1536 : (g + 1) * 1536],
                            MULT,
                            ADD,
                        )
                    else:
                        # exp on Act (psum -> sbuf bf16), then * exp(B)
                        nc.scalar.activation(p_t, s_ps, EXP)
                        eng = nc.vector if path == "A" else nc.gpsimd

                        def m_thunk(eng=eng, p_t=p_t, eb_t=eb_t, g=g):
                            eng.tensor_mul(
                                p_t, p_t, eb_t[:, g * 1536 : (g + 1) * 1536]
                            )

                        m_queue.append((gc, m_thunk))

                    if VARIANT == "core":
                        pump(gc)
                        continue

                    def o_thunk(chunks=chunks, p_t=p_t, b=b, iw=iw):
                        o_ps = get_o_ps()
                        for jc, row, off in chunks:
                            if O_COLTILE:
                                base = 64 * (jc % 2)
                                nc.tensor.matmul(
                                    o_ps[base : base + D + 1, 0:iw],
                                    v_sb[b][:, jc * (D + 1) : (jc + 1) * (D + 1)],
                                    p_t[:, off : off + iw],
                                    start=(jc < 2),
                                    stop=(jc >= NJ - 2),
                                )
                            else:
                                nc.tensor.matmul(
                                    o_ps[0 : D + 1, 0:iw],
                                    v_sb[b][:, jc * (D + 1) : (jc + 1) * (D + 1)],
                                    p_t[:, off : off + iw],
                                    start=(jc == 0),
                                    stop=(jc == NJ - 1),
                                )

                    o_queue.append((gc, o_thunk))
                    pump(gc)

                def closing(b=b, i0=i0, iw=iw):
                    o_ps = get_o_ps()
                    nrow = 97 if O_COLTILE else D + 1
                    o_t = opool.tile([128, 512], F32R, tag="ot")
                    ev_eng = getattr(nc, OT_EVAC_ENG)

                    def evac(dst, src):
                        if ev_eng is nc.scalar:
                            ev_eng.copy(dst, src)
                        else:
                            ev_eng.tensor_copy(dst, src)

                    if O_COLTILE:
                        evac(o_t[0 : D + 1, 0:iw], o_ps[0 : D + 1, 0:iw])
                        evac(o_t[64 : 64 + D + 1, 0:iw], o_ps[64 : 64 + D + 1, 0:iw])
                        nc.sync.dma_start(
                            sums_d.ap()[b, 0, i0 : i0 + iw],
                            o_t[D : D + 1, 0:iw].bitcast(F32),
                        )
                        nc.sync.dma_start(
                            sums_d.ap()[b, 1, i0 : i0 + iw],
                            o_t[96:97, 0:iw].bitcast(F32),
                        )
                    else:
                        evac(o_t[0 : D + 1, 0:iw], o_ps[0 : D + 1, 0:iw])
                        nc.sync.dma_start(
                            sums_d.ap()[b, 0, i0 : i0 + iw],
                            o_t[D : D + 1, 0:iw].bitcast(F32),
                        )
                        nc.sync.dma_start(
                            sums_d.ap()[b, 1, i0 : i0 + iw], zrow[:, 0:iw]
                        )

                    out_view = out_d.ap()[b].rearrange("(cc p) n -> p cc n", p=128)
                    for cc in range(2):
                        op_ps = psA.tile([128, 512], F32, tag="pa")
                        nc.tensor.matmul(
                            op_ps[:, 0:iw],
                            wo_r[0:nrow, cc * 128 : (cc + 1) * 128],
                            o_t[0:nrow, 0:iw],
                            start=True,
                            stop=True,
                        )
                        ev = outpool.tile([128, 512], F32, tag="ev")
                        oe = getattr(nc, OUT_EVAC_ENG)
                        if oe is nc.scalar:
                            oe.copy(ev[:, 0:iw], op_ps[:, 0:iw])
                        else:
                            oe.tensor_copy(ev[:, 0:iw], op_ps[:, 0:iw])
                        nc.sync.dma_start(
                            out_view[:, cc, i0 : i0 + iw], ev[:, 0:iw]
                        )

                if VARIANT != "core":
                    c_queue.append((gctr[0] - 1, closing))
                elif ib == len(IBLOCKS) - 1 and b == B - 1:
                    # dummy writes so outputs are bound
                    ev = outpool.tile([128, 512], F32, tag="ev")
                    nc.vector.memset(ev, 0.0)
                    for bb in range(B):
                        nc.sync.dma_start(sums_d.ap()[bb, 0, 0:512], ev[0:1, 0:512])
                        nc.sync.dma_start(sums_d.ap()[bb, 1, 0:512], ev[0:1, 0:512])
                        for cc in range(2):
                            nc.sync.dma_start(
                                out_d.ap()[bb].rearrange("(cc p) n -> p cc n", p=128)[
                                    :, cc, 0:512
                                ],
                                ev,
                            )

            # eb loading: ACT groups read exp(B) bf16 from expb, schraudolph
            # groups read int16(A*B + 16256) from ebi; both are 2-byte so
            # they share the eb_t tile (i16 slices via bitcast).
            def load_eb(ib):
                i0, iw = IBLOCKS[ib]
                eb_t = ebpool.tile([128, NJ * iw], BF16, tag="eb")
                if iw == 512:
                    srcs = {
                        "A": eb_d.ap().rearrange("(jc p) i -> p jc i", p=128),
                        "S": ebi_d.ap().rearrange("(jc p) i -> p jc i", p=128),
                    }
                    view = eb_t.rearrange("p (jc i) -> p jc i", i=iw)
                    iview = eb_t.bitcast(I16).rearrange("p (jc i) -> p jc i", i=iw)
                    # batch contiguous same-form group runs into single DMAs
                    runs = []
                    for g in range(6):
                        form = "S" if path_of(ib, g) == "D" else "A"
                        if runs and runs[-1][0] == form:
                            runs[-1][2] = 3 * (g + 1)
                        else:
                            runs.append([form, 3 * g, 3 * (g + 1)])
                    for form, lo, hi in runs:
                        dst = view if form == "A" else iview
                        nc.sync.dma_start(
                            dst[:, lo:hi, :],
                            srcs[form][:, lo:hi, i0 : i0 + iw],
                        )
                else:
                    # tail: match the bank-interleaved group layout
                    srcs = {
                        "A": eb_d.ap().rearrange(
                            "(gg u v p) i -> p gg u v i", p=128, v=3, u=2
                        ),
                        "S": ebi_d.ap().rearrange(
                            "(gg u v p) i -> p gg u v i", p=128, v=3, u=2
                        ),
                    }
                    for g in range(3):
                        form = "A" if path_of(ib, g) != "D" else "S"
                        gsl = eb_t[:, g * 1536 : (g + 1) * 1536]
                        if form == "S":
                            gsl = gsl.bitcast(I16)
                        for u in range(2):
                            nc.sync.dma_start(
                                gsl.rearrange(
                                    "p (v u i) -> p u v i", u=2, i=iw
                                )[:, u],
                                srcs[form][:, g, u, :, i0 : i0 + iw],
                            )
                return eb_t

            for _rep in range(reps):
                eb0 = load_eb(0)
                proj_batch(0)
                for ib in range(len(IBLOCKS)):
                    eb_t = eb0 if ib == 0 else load_eb(ib)
                    for b in range(B):
                        if ib == 0 and b >= 1:
                            proj_batch(b)
                        attn(b, ib, eb_t)
                flush_all()
    return nc


_CACHE = {}


def _build(reps=1):
    key = ("nc", reps, VARIANT, ASSIGN, OUT_EVAC_ENG, O_COLTILE,
           QK_EVAC_ENG, OT_EVAC_ENG, LAG_M, LAG_O, LAG_C, SPSUM_BUFS, PP_BUFS)
    if key not in _CACHE:
        nc = bacc.Bacc("TRN2", target_bir_lowering=False, debug=False, num_devices=HEADS)
        _emit(nc, reps=reps)
        nc.compile()
        _CACHE[key] = nc
    return _CACHE[key]


def _prep_inputs(x, pos_bias, w_qkv, w_out):
    bf16 = ml_dtypes.bfloat16
    xf = np.ascontiguousarray(x.reshape(B, C, N).astype(bf16))
    in_maps = []
    for h in range(HEADS):
        wq = np.ascontiguousarray(w_qkv[h * D : (h + 1) * D, :].T) * np.float32(SCALE)
        wk = np.ascontiguousarray(w_qkv[C + h * D : C + (h + 1) * D, :].T)
        wv = np.ascontiguousarray(w_qkv[2 * C + h * D : 2 * C + (h + 1) * D, :].T)
        wo = np.ascontiguousarray(w_out[:, h * D : (h + 1) * D].T)  # [32, 256]
        wo2 = np.zeros((97, C), dtype=np.float32)
        wo2[0:D] = wo
        wo2[64 : 64 + D] = wo
        bT = pos_bias[h].T.astype(np.float64)
        eb = np.exp(bT).astype(bf16)
        ebi = np.round(SCH_A * bT + SCH_B).astype(np.int16)
        in_maps.append(
            {
                "x": xf,
                "wq": np.ascontiguousarray(np.tile(wq, (1, 3))).astype(bf16),
                "wk": np.ascontiguousarray(np.tile(wk, (1, 3))).astype(bf16),
                "wv": wv.astype(bf16),
                "wo": wo2,
                "expb": np.ascontiguousarray(eb),
                "ebi": np.ascontiguousarray(ebi),
            }
        )
    return in_maps


def _run(inputs, trace=False):
    x = np.asarray(inputs["x"], dtype=np.float32)
    pos_bias = np.asarray(inputs["pos_bias"], dtype=np.float32)
    w_qkv = np.asarray(inputs["w_qkv"], dtype=np.float32)
    w_out = np.asarray(inputs["w_out"], dtype=np.float32)
    b_out = np.asarray(inputs["b_out"], dtype=np.float32)

    nc = _build()
    in_maps = _prep_inputs(x, pos_bias, w_qkv, w_out)
    res = bass_utils.run_bass_kernel_spmd(
        nc, in_maps, core_ids=list(range(HEADS)), trace=trace
    )
    out = np.zeros((B, C, N), dtype=np.float32)
    for h in range(HEADS):
        o = res.results[h]["out_un"]
        s = res.results[h]["sums"]
        out += o / (s[:, 0][:, None, :] + s[:, 1][:, None, :])
    out += b_out[None, :, None]
    return out.reshape(B, C, H, W).astype(np.float32), res


def kernel(**inputs):
    return _run(inputs)[0]


# revision 53
# speedup vs baseline: 1.1529x; 1.1529x over previous
"""Multi-head attention kernel for Trainium2, sharded one head per NeuronCore.

Math (per head h, batch b):
  q,k,v = W_{q,k,v} @ x        [32, n]   (n = 48*48 = 2304, c = 256)
  S~[j,i] = sum_d k[d,j] q[d,i]          (S transposed: j on partitions)
  P[j,i]  = exp(S~[j,i] + pos_bias[h].T[j,i])
     computed either as exp(S~)*exp(B) (Act exp + DVE mult) or via the
     Schraudolph bit-trick on DVE/GpSimd:
       bf16bits(P) ~= int16(A*S~ + round(A*B + 16256)),  A = 128/ln2
     which fuses the bias add and the exp into one off-Act instruction.
  O_ext: [v.T | 1]^T-contracted with P, 2-way column-tiled over the PE
     (even j-chunks -> psum rows 0..32, odd -> rows 64..96) so the two
     accumulation chains run concurrently on distinct col groups:
       rows 0..31/64..95 = partial attn@v (transposed), rows 32/96 = sums
  out_un[c,i] = sum_r wo2[c, r] * o_t[r, i]   (wo2 = [wo; 0; wo; 0])
Host: out = sum_h out_un_h / (sums0_h + sums1_h) + b_out.
"""

import sys

for _p in ("/opt/trn_rl_repo", "/root/.axon_site/_ro/trn_rl_repo"):
    if _p not in sys.path:
        sys.path.append(_p)

import numpy as np
import ml_dtypes

import concourse.bacc as bacc
import concourse.mybir as mybir
import concourse.tile as tile
from concourse import bass_utils

HEADS = 8
D = 32                      # dim per head
SCALE = D ** -0.5
B = 4                       # batch
C = 256                     # channels
N = 2304                    # tokens (48*48)
H = W = 48
NJ = 18                     # 128-row j-chunks
JG = 3                      # j-chunks per ACT group (3 psum banks)
NG = NJ // JG               # groups per (b, i-block)
IBLOCKS = [(0, 512), (512, 512), (1024, 512), (1536, 512), (2048, 256)]

F32 = mybir.dt.float32
F32R = mybir.dt.float32r
BF16 = mybir.dt.bfloat16
I16 = mybir.dt.int16
EXP = mybir.ActivationFunctionType.Exp
MULT = mybir.AluOpType.mult
ADD = mybir.AluOpType.add

SCH_A = float(128.0 / np.log(2.0))   # Schraudolph scale: bf16 exponent grid
# 127<<7, mean-centered (-7.37) so mixed exact/approx rows carry no relative
# bias in the softmax, +0.5 compensating the device's truncating f32->i16.
SCH_B = 16256.0 - 7.37 + 0.5

VARIANT = "full"
LAG_M = 3                  # groups of slack: exp -> eb-mult
LAG_O = 12                 # groups of slack: P final -> O matmuls
# o_t evac must emit immediately after its unit's last O matmul so the
# single o_ps bank can rotate to the next unit.
LAG_C = LAG_O
# out-projection waits LAG_P groups so its o_t input (evacuated through the
# busy Act queue) is ready before the matmul reaches the PE queue head.
LAG_P = LAG_O + 4
# tuning knobs
OUT_EVAC_ENG = "scalar"    # engine for out-projection psum->sbuf evacuation
O_COLTILE = True           # 2-way col-tiled O accumulation
# per (ib, g) group path: 'A' = Act exp + DVE mult, 'B' = Act exp + Pool
# (gpsimd) mult, 'D' = DVE schraudolph (GPSIMD cannot read PSUM, so the
# schraudolph path is DVE-only).  B groups lead each i-block: their
# two-engine chain has the longest latency, so give them the most slack
# before their deferred O matmuls reach the head of the PE queue.
ASSIGN = (
    "BDBDAA",
    "BDADAD",
    "BDBDAA",
    "BDADAD",
    "BDA",
)
QK_EVAC_ENG = "vector"     # engine for q/k psum->sbuf evacuation
OT_EVAC_ENG = "scalar"     # engine for o_t psum->sbuf evacuation
SPSUM_BUFS = 2             # S-psum group double buffering (3 banks each)
PP_BUFS = 15               # P-tile pool depth


def _emit(nc, reps=1):
    x_d = nc.dram_tensor("x", [B, C, N], BF16, kind="ExternalInput")
    wq_d = nc.dram_tensor("wq", [C, 96], BF16, kind="ExternalInput")
    wk_d = nc.dram_tensor("wk", [C, 96], BF16, kind="ExternalInput")
    wv_d = nc.dram_tensor("wv", [C, D], BF16, kind="ExternalInput")
    wo_d = nc.dram_tensor("wo", [97, C], F32, kind="ExternalInput")
    eb_d = nc.dram_tensor("expb", [N, N], BF16, kind="ExternalInput")
    ebi_d = nc.dram_tensor("ebi", [N, N], I16, kind="ExternalInput")
    out_d = nc.dram_tensor("out_un", [B, C, N], F32, kind="ExternalOutput")
    sums_d = nc.dram_tensor("sums", [B, 2, N], F32, kind="ExternalOutput")

    def path_of(ib, g):
        return ASSIGN[ib][g]

    with tile.TileContext(nc) as tc:
        with (
            tc.tile_pool(name="wpool", bufs=1) as wpool,
            tc.tile_pool(name="qk", bufs=8) as qkpool,
            tc.tile_pool(name="vext", bufs=4) as vpool,
            tc.tile_pool(name="big", bufs=2) as bigpool,
            tc.tile_pool(name="pp", bufs=PP_BUFS) as ppool,
            tc.tile_pool(name="ebpool", bufs=2) as ebpool,
            tc.tile_pool(name="outsb", bufs=3) as outpool,
            tc.tile_pool(name="osb", bufs=2) as opool,
            tc.tile_pool(name="spsum", bufs=SPSUM_BUFS, space="PSUM") as spsum,
            tc.tile_pool(name="psO", bufs=1, space="PSUM") as psO,
            tc.tile_pool(name="psP", bufs=1, space="PSUM") as psP,
        ):
            # ---- weights: bf16 straight from HBM (host pre-converts) ----
            w_r = {}
            for name, dram, shape in (
                ("wq", wq_d, [128, 2, 96]),
                ("wk", wk_d, [128, 2, 96]),
                ("wv", wv_d, [128, 2, D]),
            ):
                raw = wpool.tile(shape, BF16, tag=f"{name}raw")
                nc.sync.dma_start(raw, dram.ap().rearrange("(cc p) m -> p cc m", p=128))
                w_r[name] = raw
            wo_raw = wpool.tile([97, C], F32, tag="woraw")
            nc.sync.dma_start(wo_raw, wo_d.ap())
            wo_r = wpool.tile([97, C], F32R, tag="wor")
            nc.vector.tensor_copy(wo_r, wo_raw)
            zrow = wpool.tile([1, 512], F32, tag="zrow")
            nc.vector.memset(zrow, 0.0)
            # o_t buffers: zero once so the dead partition band (33..63)
            # contributes exact zeros to the K=97 out-projection.
            for _ in range(2):
                t = opool.tile([128, 512], F32R, tag="ot")
                nc.vector.memset(t.bitcast(F32), 0.0)

            # ---- phase 0 per batch: load x, project q/k/v ----
            q_sb = [None] * B
            k_sb = [None] * B
            v_sb = [None] * B

            def proj_batch(b):
                x_r = bigpool.tile([128, 2, N], BF16, tag="big")
                x_view = x_d.ap()[b].rearrange("(cc p) n -> p cc n", p=128)
                for cc in range(2):
                    nc.sync.dma_start(x_r[:, cc, :], x_view[:, cc, :])

                # q and k replicated 3x along output rows (for PE row-tiling)
                for name, store in (("wk", k_sb), ("wq", q_sb)):
                    dst = qkpool.tile([128, N], F32R, tag="qk")
                    store[b] = dst
                    for ti, islices in ((0, (0, 1, 2)), (1, (3, 4))):
                        pt = spsum.tile([128, 3 * 512], F32, tag="sg")
                        for sl, ic in enumerate(islices):
                            i0, iw = IBLOCKS[ic]
                            for cc in range(2):
                                nc.tensor.matmul(
                                    pt[0:96, sl * 512 : sl * 512 + iw],
                                    w_r[name][:, cc, :],
                                    x_r[:, cc, i0 : i0 + iw],
                                    start=(cc == 0),
                                    stop=(cc == 1),
                                )
                        nw = sum(IBLOCKS[ic][1] for ic in islices)
                        eng = nc.scalar if b < 1 else getattr(nc, QK_EVAC_ENG)
                        if eng is nc.scalar:
                            eng.copy(
                                dst[0:96, ti * 1536 : ti * 1536 + nw], pt[0:96, 0:nw]
                            )
                        else:
                            eng.tensor_copy(
                                dst[0:96, ti * 1536 : ti * 1536 + nw], pt[0:96, 0:nw]
                            )

                # v transposed directly: v_T[n, d] = x^T @ wv_T, 18 chunks
                vext = vpool.tile([128, NJ * (D + 1)], BF16, tag="vext")
                v_sb[b] = vext
                nc.vector.memset(vext, 1.0)
                vt = spsum.tile([128, 3 * 512], F32, tag="sg")
                for jc in range(NJ):
                    for cc in range(2):
                        nc.tensor.matmul(
                            vt[:, jc * D : (jc + 1) * D],
                            x_r[:, cc, jc * 128 : (jc + 1) * 128],
                            w_r["wv"][:, cc, :],
                            start=(cc == 0),
                            stop=(cc == 1),
                        )
                nc.vector.tensor_copy(
                    vext.rearrange("p (jc m) -> p jc m", m=D + 1)[:, :, 0:D],
                    vt.rearrange("p (jc m) -> p jc m", m=D)[:, 0:NJ, :],
                )

            # Deferred-emission queues.  Every engine queue is strict FIFO
            # on hardware, so an instruction whose inputs aren't ready
            # head-of-line-blocks everything behind it on that engine.
            # Emit each dependent stage LAG groups after its producer so by
            # the time it reaches its engine's queue head, the input exists:
            #   m_queue: A/B-path eb-multiplies (wait on Act exp)
            #   o_queue: O matmuls (wait on final P)
            #   c_queue: per-(b,ib) o_t evac + sums DMA (wait on O matmuls)
            #   p_queue: per-(b,ib) out-projection (waits on o_t evac)
            m_queue, o_queue, c_queue, p_queue = [], [], [], []
            gctr = [0]
            lagged = ((m_queue, LAG_M), (o_queue, LAG_O),
                      (c_queue, LAG_C), (p_queue, LAG_P))

            def pump(cur):
                for q, lag in lagged:
                    while q and q[0][0] <= cur - lag:
                        q.pop(0)[1]()

            def flush_all():
                # drain in global group order (ties follow the stage order
                # above) so pool buffer reuse stays emission-ordered
                while any(q for q, _ in lagged):
                    cands = [
                        (q[0][0], i, q)
                        for i, (q, _) in enumerate(lagged)
                        if q
                    ]
                    cands.sort()
                    cands[0][2].pop(0)[1]()

            def group_layout(iw):
                """Per ACT-group chunk placement in the 3-bank S tile."""
                if iw == 512:
                    return [[(g * 3 + jl, jl, jl * 512) for jl in range(3)]
                            for g in range(6)]
                return [
                    [(g * 6 + c, c % 3, (c % 3) * 512 + (c // 3) * 256)
                     for c in range(6)]
                    for g in range(3)
                ]

            def attn(b, ib, eb_t):
                i0, iw = IBLOCKS[ib]
                # o_ps is allocated lazily inside the first deferred O-thunk
                # so psA buffer reuse follows emission order (no deadlock).
                o_ps_box = []

                def get_o_ps():
                    if not o_ps_box:
                        o_ps_box.append(
                            psO.tile([128, 512], F32, tag="po", name="o_ps")
                        )
                    return o_ps_box[0]

                for g, chunks in enumerate(group_layout(iw)):
                    path = path_of(ib, g)
                    s_ps = spsum.tile([128, 3 * 512], F32, tag="sg")
                    for jc, row, off in chunks:
                        nc.tensor.matmul(
                            s_ps[:, off : off + iw],
                            k_sb[b][32 * row : 32 * row + 32, jc * 128 : (jc + 1) * 128],
                            q_sb[b][32 * row : 32 * row + 32, i0 : i0 + iw],
                            start=True,
                            stop=True,
                        )
                    p_t = ppool.tile([128, 3 * 512], BF16, tag="pt")
                    gc = gctr[0]
                    gctr[0] += 1
                    if path == "D":
                        # Schraudolph: bf16bits = int16(A*S + ebi), fused
                        # bias add + exp approx, off the Act engine.
                        nc.vector.scalar_tensor_tensor(
                            p_t.bitcast(I16),
                            s_ps,
                            SCH_A,
                            eb_t.bitcast(I16)[:, g * 1536 : (g + 1) * 1536],
                            MULT,
                            ADD,
                        )
                    else:
                        # exp on Act (psum -> sbuf bf16), then * exp(B)
                        nc.scalar.activation(p_t, s_ps, EXP)
                        eng = nc.vector if path == "A" else nc.gpsimd

                        def m_thunk(eng=eng, p_t=p_t, eb_t=eb_t, g=g):
                            eng.tensor_mul(
                                p_t, p_t, eb_t[:, g * 1536 : (g + 1) * 1536]
                            )

                        m_queue.append((gc, m_thunk))

                    if VARIANT == "core":
                        pump(gc)
                        continue

                    def o_thunk(chunks=chunks, p_t=p_t, b=b, iw=iw):
                        o_ps = get_o_ps()
                        for jc, row, off in chunks:
                            if O_COLTILE:
                                base = 64 * (jc % 2)
                                nc.tensor.matmul(
                                    o_ps[base : base + D + 1, 0:iw],
                                    v_sb[b][:, jc * (D + 1) : (jc + 1) * (D + 1)],
                                    p_t[:, off : off + iw],
                                    start=(jc < 2),
                                    stop=(jc >= NJ - 2),
                                )
                            else:
                                nc.tensor.matmul(
                                    o_ps[0 : D + 1, 0:iw],
                                    v_sb[b][:, jc * (D + 1) : (jc + 1) * (D + 1)],
                                    p_t[:, off : off + iw],
                                    start=(jc == 0),
                                    stop=(jc == NJ - 1),
                                )

                    o_queue.append((gc, o_thunk))
                    pump(gc)

                def closing(b=b, i0=i0, iw=iw, gc_unit=gctr[0] - 1):
                    o_ps = get_o_ps()
                    nrow = 97 if O_COLTILE else D + 1
                    o_t = opool.tile([128, 512], F32R, tag="ot")
                    ev_eng = getattr(nc, OT_EVAC_ENG)

                    def evac(dst, src):
                        if ev_eng is nc.scalar:
                            ev_eng.copy(dst, src)
                        else:
                            ev_eng.tensor_copy(dst, src)

                    if O_COLTILE:
                        evac(o_t[0 : D + 1, 0:iw], o_ps[0 : D + 1, 0:iw])
                        evac(o_t[64 : 64 + D + 1, 0:iw], o_ps[64 : 64 + D + 1, 0:iw])
                        nc.sync.dma_start(
                            sums_d.ap()[b, 0, i0 : i0 + iw],
                            o_t[D : D + 1, 0:iw].bitcast(F32),
                        )
                        nc.sync.dma_start(
                            sums_d.ap()[b, 1, i0 : i0 + iw],
                            o_t[96:97, 0:iw].bitcast(F32),
                        )
                    else:
                        evac(o_t[0 : D + 1, 0:iw], o_ps[0 : D + 1, 0:iw])
                        nc.sync.dma_start(
                            sums_d.ap()[b, 0, i0 : i0 + iw],
                            o_t[D : D + 1, 0:iw].bitcast(F32),
                        )
                        nc.sync.dma_start(
                            sums_d.ap()[b, 1, i0 : i0 + iw], zrow[:, 0:iw]
                        )

                    def outproj(b=b, i0=i0, iw=iw, o_t=o_t, nrow=nrow):
                        out_view = out_d.ap()[b].rearrange(
                            "(cc p) n -> p cc n", p=128
                        )
                        for cc in range(2):
                            op_ps = psP.tile(
                                [128, 512], F32, tag="pp", name="op_ps"
                            )
                            nc.tensor.matmul(
                                op_ps[:, 0:iw],
                                wo_r[0:nrow, cc * 128 : (cc + 1) * 128],
                                o_t[0:nrow, 0:iw],
                                start=True,
                                stop=True,
                            )
                            ev = outpool.tile([128, 512], F32, tag="ev")
                            oe = getattr(nc, OUT_EVAC_ENG)
                            if oe is nc.scalar:
                                oe.copy(ev[:, 0:iw], op_ps[:, 0:iw])
                            else:
                                oe.tensor_copy(ev[:, 0:iw], op_ps[:, 0:iw])
                            nc.sync.dma_start(
                                out_view[:, cc, i0 : i0 + iw], ev[:, 0:iw]
                            )

                    p_queue.append((gc_unit, outproj))

                if VARIANT != "core":
                    c_queue.append((gctr[0] - 1, closing))
                elif ib == len(IBLOCKS) - 1 and b == B - 1:
                    # dummy writes so outputs are bound
                    ev = outpool.tile([128, 512], F32, tag="ev")
                    nc.vector.memset(ev, 0.0)
                    for bb in range(B):
                        nc.sync.dma_start(sums_d.ap()[bb, 0, 0:512], ev[0:1, 0:512])
                        nc.sync.dma_start(sums_d.ap()[bb, 1, 0:512], ev[0:1, 0:512])
                        for cc in range(2):
                            nc.sync.dma_start(
                                out_d.ap()[bb].rearrange("(cc p) n -> p cc n", p=128)[
                                    :, cc, 0:512
                                ],
                                ev,
                            )

            # eb loading: ACT groups read exp(B) bf16 from expb, schraudolph
            # groups read int16(A*B + 16256) from ebi; both are 2-byte so
            # they share the eb_t tile (i16 slices via bitcast).
            def load_eb(ib):
                i0, iw = IBLOCKS[ib]
                eb_t = ebpool.tile([128, NJ * iw], BF16, tag="eb")
                if iw == 512:
                    srcs = {
                        "A": eb_d.ap().rearrange("(jc p) i -> p jc i", p=128),
                        "S": ebi_d.ap().rearrange("(jc p) i -> p jc i", p=128),
                    }
                    view = eb_t.rearrange("p (jc i) -> p jc i", i=iw)
                    iview = eb_t.bitcast(I16).rearrange("p (jc i) -> p jc i", i=iw)
                    # batch contiguous same-form group runs into single DMAs
                    runs = []
                    for g in range(6):
                        form = "S" if path_of(ib, g) == "D" else "A"
                        if runs and runs[-1][0] == form:
                            runs[-1][2] = 3 * (g + 1)
                        else:
                            runs.append([form, 3 * g, 3 * (g + 1)])
                    for form, lo, hi in runs:
                        dst = view if form == "A" else iview
                        nc.sync.dma_start(
                            dst[:, lo:hi, :],
                            srcs[form][:, lo:hi, i0 : i0 + iw],
                        )
                else:
                    # tail: match the bank-interleaved group layout
                    srcs = {
                        "A": eb_d.ap().rearrange(
                            "(gg u v p) i -> p gg u v i", p=128, v=3, u=2
                        ),
                        "S": ebi_d.ap().rearrange(
                            "(gg u v p) i -> p gg u v i", p=128, v=3, u=2
                        ),
                    }
                    for g in range(3):
                        form = "A" if path_of(ib, g) != "D" else "S"
                        gsl = eb_t[:, g * 1536 : (g + 1) * 1536]
                        if form == "S":
                            gsl = gsl.bitcast(I16)
                        for u in range(2):
                            nc.sync.dma_start(
                                gsl.rearrange(
                                    "p (v u i) -> p u v i", u=2, i=iw
                                )[:, u],
                                srcs[form][:, g, u, :, i0 : i0 + iw],
                            )
                return eb_t

            for _rep in range(reps):
                eb0 = load_eb(0)
                proj_batch(0)
                for ib in range(len(IBLOCKS)):
                    eb_t = eb0 if ib == 0 else load_eb(ib)
                    for b in range(B):
                        if ib == 0 and b >= 1:
                            proj_batch(b)
                        attn(b, ib, eb_t)
                flush_all()
    return nc


_CACHE = {}


def _build(reps=1):
    key = ("nc", reps, VARIANT, ASSIGN, OUT_EVAC_ENG, O_COLTILE,
           QK_EVAC_ENG, OT_EVAC_ENG, LAG_M, LAG_O, LAG_C, LAG_P, SPSUM_BUFS, PP_BUFS)
    if key not in _CACHE:
        nc = bacc.Bacc("TRN2", target_bir_lowering=False, debug=False, num_devices=HEADS)
        _emit(nc, reps=reps)
        nc.compile()
        _CACHE[key] = nc
    return _CACHE[key]


def _prep_inputs(x, pos_bias, w_qkv, w_out):
    bf16 = ml_dtypes.bfloat16
    xf = np.ascontiguousarray(x.reshape(B, C, N).astype(bf16))
    in_maps = []
    for h in range(HEADS):
        wq = np.ascontiguousarray(w_qkv[h * D : (h + 1) * D, :].T) * np.float32(SCALE)
        wk = np.ascontiguousarray(w_qkv[C + h * D : C + (h + 1) * D, :].T)
        wv = np.ascontiguousarray(w_qkv[2 * C + h * D : 2 * C + (h + 1) * D, :].T)
        wo = np.ascontiguousarray(w_out[:, h * D : (h + 1) * D].T)  # [32, 256]
        wo2 = np.zeros((97, C), dtype=np.float32)
        wo2[0:D] = wo
        wo2[64 : 64 + D] = wo
        bT = pos_bias[h].T.astype(np.float64)
        eb = np.exp(bT).astype(bf16)
        ebi = np.round(SCH_A * bT + SCH_B).astype(np.int16)
        in_maps.append(
            {
                "x": xf,
                "wq": np.ascontiguousarray(np.tile(wq, (1, 3))).astype(bf16),
                "wk": np.ascontiguousarray(np.tile(wk, (1, 3))).astype(bf16),
                "wv": wv.astype(bf16),
                "wo": wo2,
                "expb": np.ascontiguousarray(eb),
                "ebi": np.ascontiguousarray(ebi),
            }
        )
    return in_maps


def _run(inputs, trace=False):
    x = np.asarray(inputs["x"], dtype=np.float32)
    pos_bias = np.asarray(inputs["pos_bias"], dtype=np.float32)
    w_qkv = np.asarray(inputs["w_qkv"], dtype=np.float32)
    w_out = np.asarray(inputs["w_out"], dtype=np.float32)
    b_out = np.asarray(inputs["b_out"], dtype=np.float32)

    nc = _build()
    in_maps = _prep_inputs(x, pos_bias, w_qkv, w_out)
    res = bass_utils.run_bass_kernel_spmd(
        nc, in_maps, core_ids=list(range(HEADS)), trace=trace
    )
    out = np.zeros((B, C, N), dtype=np.float32)
    for h in range(HEADS):
        o = res.results[h]["out_un"]
        s = res.results[h]["sums"]
        out += o / (s[:, 0][:, None, :] + s[:, 1][:, None, :])
    out += b_out[None, :, None]
    return out.reshape(B, C, H, W).astype(np.float32), res


def kernel(**inputs):
    return _run(inputs)[0]
